# revision 4
# baseline (speedup 1.0000x reference)
"""Blockwise 8x8 DCT + int8 quantization (compressor) on 8 trn2 NeuronCores.

Strategy (pure data parallel, zero communication):
- Shard x row-wise: core c gets rows [1024c, 1024c+1024) = 128 block-rows
  = 131072 blocks of 8x8.
- Host pre-layout per core: xr[128, 65536] fp32 where rows 0-63 hold the
  flattened (ef) components of blocks [0, 65536) (one block per column) and
  rows 64-127 hold blocks [65536, 131072).
- Device: one K=128 matmul per 256 blocks with the stationary operand being
  the data chunk [128ef, 128b] and the moving operand K2 = blockdiag(kron(D,D),
  kron(D,D)) [128, 128].  out[p, 64h+k] = coeff k of block (65536h + base + p):
  coefficients are born in [block-partition, coeff-free] layout, so the
  per-block abs-max is a free-dim reduce and quantization is a per-partition
  tensor_scalar / activation-scale op.  No transposes anywhere.
- biggest: DVE tensor_reduce(max, abs) straight from PSUM (fp32 exact).
- indices: round(coeff * (1/biggest) * 127) -> int8 (RNE convert, matches
  jnp.round bit-for-bit up to fp32 reassociation noise), split between the
  Scalar (activation with per-partition scale) and Vector (tensor_scalar with
  two scalar operands) engines.
- Host post: undo the block permutation and stack core shards.
"""

import os

# The axon client in this container has no NTFF hook (stub antenv); make sure
# nothing tries to trace through it.
os.environ.setdefault("BASS_NEVER_TRACE", "1")

import numpy as np
from contextlib import ExitStack

import concourse.bacc as bacc
import concourse.tile as tile
from concourse import mybir
from concourse.bass_utils import run_bass_kernel_spmd

F32 = mybir.dt.float32
I8 = mybir.dt.int8

N_CORES = 8
H = W = 8192
BLOCK = 8
ROWS_PER_CORE = H // N_CORES              # 1024
NB_CORE = (ROWS_PER_CORE // BLOCK) * (W // BLOCK)  # 131072 blocks per core
NBH = NB_CORE // 2                        # 65536 blocks per half
NG = NBH // 512                           # 128 groups (1024 blocks each)
IN_GROUPS = 4                             # groups per input DMA  -> [128, 2048] f32 = 1 MiB
GI = 4                                    # groups per idx output DMA -> [128, 2048] i8
GB = 16                                   # groups per big output DMA -> [128, 128] f32
N_ACT = 5                                 # quantize chunks on ScalarE (rest on VectorE)

_CACHE: dict = {}


def _build():
    nc = bacc.Bacc(None, target_bir_lowering=False)
    xin = nc.declare_dram_parameter("xr", [128, NBH], F32, isOutput=False)
    k2in = nc.declare_dram_parameter("k2", [128, 128], F32, isOutput=False)
    dev_idx = nc.declare_dram_parameter("idx", [128, NBH], I8, isOutput=True)
    dev_big = nc.declare_dram_parameter("big", [128, NG * 8], F32, isOutput=True)

    with tile.TileContext(nc) as tc:
        with ExitStack() as ctx:
            const_pool = ctx.enter_context(tc.tile_pool(name="const", bufs=1))
            xpool = ctx.enter_context(tc.tile_pool(name="xin", bufs=3))
            pspool = ctx.enter_context(tc.tile_pool(name="ps", bufs=4, space="PSUM"))
            idxpool = ctx.enter_context(tc.tile_pool(name="idx", bufs=3))
            bigpool = ctx.enter_context(tc.tile_pool(name="big", bufs=2))
            recpool = ctx.enter_context(tc.tile_pool(name="rec", bufs=4))

            k2_t = const_pool.tile([128, 128], F32)
            nc.sync.dma_start(k2_t[:], k2in[:])

            x_t = None
            idx_b = None
            big_b = None
            for g in range(NG):
                if g % IN_GROUPS == 0:
                    x_t = xpool.tile([128, 512 * IN_GROUPS], F32, tag="x")
                    nc.sync.dma_start(
                        x_t[:], xin[:, 512 * g : 512 * (g + IN_GROUPS)]
                    )
                if g % GI == 0:
                    idx_b = idxpool.tile([128, 512 * GI], I8, tag="ib")
                if g % GB == 0:
                    big_b = bigpool.tile([128, 8 * GB], F32, tag="bb")

                xoff = 512 * (g % IN_GROUPS)
                psum = pspool.tile([128, 512], F32, tag="ps")
                for m in range(4):
                    nc.tensor.matmul(
                        psum[:, 128 * m : 128 * (m + 1)],
                        x_t[:, xoff + 128 * m : xoff + 128 * (m + 1)],
                        k2_t[:],
                        start=True,
                        stop=True,
                    )

                bslice = big_b[:, 8 * (g % GB) : 8 * (g % GB) + 8]
                nc.vector.tensor_reduce(
                    bslice,
                    psum[:].rearrange("p (c k) -> p c k", k=64),
                    axis=mybir.AxisListType.X,
                    op=mybir.AluOpType.max,
                    apply_absolute_value=True,
                )
                recip = recpool.tile([128, 8], F32, tag="r")
                nc.vector.reciprocal(recip[:], bslice)
                recip127 = recpool.tile([128, 8], F32, tag="r127")
                nc.vector.tensor_scalar(
                    recip127[:], recip[:], 127.0, None, op0=mybir.AluOpType.mult
                )

                ioff = 512 * (g % GI)
                for c in range(8):
                    dst = idx_b[:, ioff + 64 * c : ioff + 64 * (c + 1)]
                    src = psum[:, 64 * c : 64 * (c + 1)]
                    if c < N_ACT:
                        nc.scalar.activation(
                            dst,
                            src,
                            mybir.ActivationFunctionType.Copy,
                            scale=recip127[:, c : c + 1],
                        )
                    else:
                        nc.vector.tensor_scalar(
                            dst,
                            src,
                            recip[:, c : c + 1],
                            127.0,
                            op0=mybir.AluOpType.mult,
                            op1=mybir.AluOpType.mult,
                        )

                if g % GI == GI - 1:
                    nc.sync.dma_start(
                        dev_idx[:, 512 * (g - GI + 1) : 512 * (g + 1)], idx_b[:]
                    )
                if g % GB == GB - 1:
                    nc.sync.dma_start(
                        dev_big[:, 8 * (g - GB + 1) : 8 * (g + 1)], big_b[:]
                    )
    nc.compile()
    return nc


def _get_nc():
    if "nc" not in _CACHE:
        _CACHE["nc"] = _build()
    return _CACHE["nc"]


def kernel(x: np.ndarray, dct: np.ndarray):
    x = np.asarray(x, dtype=np.float32)
    dct = np.asarray(dct, dtype=np.float32)
    assert x.shape == (H, W)

    k64 = np.kron(dct, dct).astype(np.float32)  # [ef, gh]
    k2 = np.zeros((128, 128), np.float32)
    k2[0:64, 0:64] = k64
    k2[64:128, 64:128] = k64

    in_maps = []
    for c in range(N_CORES):
        xc = x[ROWS_PER_CORE * c : ROWS_PER_CORE * (c + 1)]
        # xb[ef, b] with b = b1_local * 1024 + b2
        xb = (
            xc.reshape(ROWS_PER_CORE // BLOCK, BLOCK, W // BLOCK, BLOCK)
            .transpose(1, 3, 0, 2)
            .reshape(64, NB_CORE)
        )
        xr = np.empty((128, NBH), np.float32)
        xr[0:64] = xb[:, :NBH]
        xr[64:128] = xb[:, NBH:]
        in_maps.append({"xr": xr, "k2": k2})

    nc = _get_nc()
    res = run_bass_kernel_spmd(nc, in_maps, core_ids=list(range(N_CORES)))
    _CACHE["last_results"] = res
    _CACHE["last_in_maps"] = in_maps

    indices = np.empty((H // BLOCK, W // BLOCK, 64), np.int8)
    biggest = np.empty((H // BLOCK, W // BLOCK), np.float32)
    bpc = ROWS_PER_CORE // BLOCK  # block-rows per core (128)
    for c in range(N_CORES):
        r = res.results[c]
        # idx[p, 512g + 64*(2m+h) + k] -> block 65536h + 512g + 128m + p
        arr = r["idx"].reshape(128, NG, 4, 2, 64)          # [p, g, m, h, k]
        blocks = arr.transpose(3, 1, 2, 0, 4).reshape(NB_CORE, 64)
        indices[bpc * c : bpc * (c + 1)] = blocks.reshape(bpc, W // BLOCK, 64)
        bb = r["big"].reshape(128, NG, 4, 2)               # [p, g, m, h]
        bflat = bb.transpose(3, 1, 2, 0).reshape(NB_CORE)
        biggest[bpc * c : bpc * (c + 1)] = bflat.reshape(bpc, W // BLOCK)

    return indices, biggest
